# revision 21
# baseline (speedup 1.0000x reference)
"""MoE expert FFN (swiglu) kernel for 8 trn2 NeuronCores.

Expert parallelism: 8 experts, one per core. Each core computes, for its
expert e:
    h   = x_e @ w1_e            # [2048, 2048] @ [2048, 2816]
    act = silu(h[:, :1408]) * h[:, 1408:]
    out = act @ w2_e            # [2048, 1408] @ [1408, 2048]

Tokens arrive pre-sorted by expert with equal counts (2048/expert), so
sharding is a static slice and the gather is a concat. No collectives.

Steady state (measured): back-to-back N=512 bf16 matmuls at 216 ns issue
spacing with LDWEIGHTS fully hidden -> 2112 MMs ~= 456 us dense PE floor.
Lower-precision matmul paths (fp8 DoubleRow 1.44x, int8) were ruled out
numerically: any <=2-matmul e4m3/int8 decomposition has >=2% rel err
against the 2e-2 gate (measured 6.5% all-e4m3, 2.1% all-int8), and
3-matmul hi/lo splits cost 2.08x bf16. So everything attacks the edges
around the dense stream (baseline 516.9 us -> ~477 us):

  1. w1 is host-repacked into 11 swiglu-pair slices [128, 16*256]
     (pair j holds the a-column-block j and b-column-block j+11 of all
     16 k-tiles, 1 MB each, contiguous 8 KB per partition). mm1 pair 0
     starts after ~1 MB of DMA instead of the whole 11.5 MB of w1
     (the original kernel idled 16 us, then stalled 17.6 us on w1
     residency). jp0 is further split in half (k-tiles 0-7 / 8-15).
  2. ALL DMA triggers live on the SP (sync) queue; the ACT queue holds
     only silu + PSUM-evacuation copies. HWDGE trigger instructions
     wait for their completion-semaphore lane's previous DMA before
     issuing, so triggers queued ahead of silu on ACT block the swiglu
     drain (measured: first silu at ~60 us -> PSUM-bank WAR stalled
     the PE 15.9 us and HAM re-throttled twice). Splitting the early
     triggers across SP+ACT rings was also measured 2-4 us WORSE (both
     rings share the same 16 SDMA engines).
  3. x is host-repacked per (chunk, quarter) so each chunk is 4
     contiguous [128, 2048] DMAs (4 KB/partition) instead of 16 small
     ones: fewer triggers, fewer sem-lane-reuse waits. Critical-path
     DMA order: jp0a, x-quarters 0-3, jp0b, jp1..jp10, w2 — pair-0's
     a-half consumes all four x quarters, jp0b only ~3.5 us later.
  4. PE pre-warm: 12 dummy matmuls on a zeroed tile right after the
     engine preamble, so the HAM clock-gate (1.2 -> 2.4 GHz after
     ~3.4 us of sustained activity) is released before the first real
     matmul, which is data-gated at ~13 us. The stream end is bound by
     delivery of chunk-0's x (2 MB) + jp0 (1 MB) at ~358 GB/s plus the
     dense 456 us stream; remaining losses are ~2 us of early
     delivery-bound gaps, ~3 us output tail, and ~7 us of framework
     semaphore-clear teardown inside the measured window.
  5. mm2 runs n-outer/k-inner so each PSUM bank finishes its 11-MM
     accumulation early and drains (ACT copy -> DMA) while the next
     bank accumulates; output staged as bf16 (host upcasts to f32,
     ~0.2% extra rounding, total rel err 4.4e-3 vs the 2e-2 gate).
     The out-DMA tail after the last matmul is ~3 us vs 14.4 us for
     the baseline's k-outer ordering with fp32 staging.

Device-side layout (bf16 compute, fp32 PSUM accumulation, bf16 out):
  mm1: out[f, t] tiles; lhsT = w1 pair-slice [128, 256] col pair
       (stationary), rhs = xT[h, t] (moving, N=512).
  swiglu: act_j = silu(ps_a) * ps_b via ACT(Silu) + DVE mul -> bf16.
  mm2: out[t, h]; lhsT = act[f, t] 128-col slices, rhs = w2[f, h]
       (moving, N=512). PSUM -> SBUF bf16 -> DMA out.
"""

import os
import sys

sys.path.insert(0, "/opt/trn_rl_repo")

import numpy as np
import ml_dtypes

E = 8             # experts == cores
T_TOTAL = 16384
H = 2048
F = 1408
F2 = 2 * F        # 2816
TPC = T_TOTAL // E  # 2048 tokens per core
CHUNK = 512
NCH = TPC // CHUNK          # 4 chunks
KH = H // 128               # 16 contraction tiles for mm1
NF = F // 128               # 11 f-blocks per half (a / b)
NT = CHUNK // 128           # 4 m-tiles per chunk in mm2
NHO = H // 512              # 4 output column blocks
NWARM = 12                  # dummy matmuls to release the HAM clock gate

_CACHE = {}

# Optional knobs read by test.py (not used by the grading harness).
TRACE = os.environ.get("BASS_TRACE_KERNEL", "0") == "1"
LAST = {}


def _build():
    from concourse import bacc, tile, mybir

    bf16 = mybir.dt.bfloat16
    f32 = mybir.dt.float32
    SILU = mybir.ActivationFunctionType.Silu

    # Bacc (not plain Bass): its lowering pipeline splits multi-sem waits
    # into EventSemaphore pairs — TRN2 allows at most 1 wait per instruction.
    nc = bacc.Bacc()
    # x packed by (chunk, quarter): row (c*4+q)*128+p, col kk*512+cc holds
    # xT[(q*4+kk)*128+p, c*512+cc] — each chunk is 4 contiguous 512 KB DMAs.
    xq_d = nc.declare_dram_parameter("xq", [NCH * 4 * 128, 4 * CHUNK], bf16, isOutput=False)
    # w1 packed by swiglu pair: row jp*128+p, col k*256 + half*128 + c
    w1_d = nc.declare_dram_parameter("w1p", [NF * 128, KH * 256], bf16, isOutput=False)
    w2_d = nc.declare_dram_parameter("w2", [F, H], bf16, isOutput=False)
    out_d = nc.declare_dram_parameter("out", [TPC, H], bf16, isOutput=True)

    with tile.TileContext(nc) as tc:
        with (
            tc.tile_pool(name="w1p", bufs=1) as w1p,
            tc.tile_pool(name="w2p", bufs=1) as w2p,
            tc.tile_pool(name="xp", bufs=2) as xp,
            tc.tile_pool(name="actp", bufs=1) as actp,
            tc.tile_pool(name="tmpp", bufs=2) as tmpp,
            tc.tile_pool(name="outp", bufs=4) as outp,
            tc.tile_pool(name="warmp", bufs=1) as warmp,
            tc.tile_pool(name="psp", bufs=8, space="PSUM") as psp,
        ):
            # PE warm-up: zero tile, then dummy matmuls with no DMA deps so
            # the PE is busy during the startup DMA window and the HAM gate
            # opens (K=8/8) before the first real matmul.
            wz = warmp.tile([128, 512], bf16, tag="wz")
            nc.vector.memset(wz[:], 0)
            ps_w = psp.tile([128, 512], f32, tag="ps", name="warm")
            for i in range(NWARM):
                nc.tensor.matmul(
                    ps_w[:],
                    wz[:, 0:128],
                    wz[:],
                    start=(i == 0),
                    stop=(i == NWARM - 1),
                )

            # SP queue order: jp0 first (gates the first real matmul), then
            # chunk 0's x quarters, then the rest of w1, then w2 (needed
            # ~100 us in, must not steal startup bandwidth).
            # x rides the SWDGE (gpsimd/Q7) path: a separate descriptor-
            # generation engine feeding different internal SDMA queues than
            # the HWDGE ring, so the w1 slice stream on SP is not serialized
            # behind x — jp1 lands ~7 us earlier, removing the pair-1 stall.
            def load_x_chunk(c):
                qt = []
                for q in range(4):
                    t = xp.tile([128, 4 * CHUNK], bf16, tag=f"x_{q}", name=f"x_{c}_{q}")
                    qt.append(t)
                    r = (c * 4 + q) * 128
                    nc.gpsimd.dma_start(out=t[:], in_=xq_d[r : r + 128, :])
                # x_t[k] is a 512-col view into quarter k//4
                return [qt[k // 4][:, (k % 4) * CHUNK : (k % 4 + 1) * CHUNK]
                        for k in range(KH)]

            # jp0 is split in half (k-tiles 0-7 / 8-15) so the first real
            # matmul is gated by ~1 MB of DMA, not 2.5 MB. Everything stays
            # on the SP queue: an A/B
            # splitting these across SP+ACT rings measured 2-4 us WORSE
            # (both rings share the same 16 SDMA engines), and triggers on
            # ACT risk stalling the swiglu drain (the v2 lesson).
            # Pair-0's a-half k-loop consumes all 4 x quarters, so they all
            # precede jp0b (the b-half weights, not needed until ~3.5 us
            # after the a-half starts completing).
            w1_0a = w1p.tile([128, 8 * 256], bf16, tag="w1_0a")
            nc.sync.dma_start(out=w1_0a[:], in_=w1_d[0:128, 0 : 8 * 256])

            x0_t = load_x_chunk(0)

            w1_0b = w1p.tile([128, 8 * 256], bf16, tag="w1_0b")
            nc.sync.dma_start(out=w1_0b[:], in_=w1_d[0:128, 8 * 256 :])

            w1_t = [None]
            for j in range(1, NF):
                t = w1p.tile([128, KH * 256], bf16, tag=f"w1_{j}")
                w1_t.append(t)
                nc.sync.dma_start(out=t[:], in_=w1_d[j * 128 : (j + 1) * 128, :])

            def w1_lhsT(j, k, half):
                if j == 0:
                    t = w1_0a if k < 8 else w1_0b
                    kk = k if k < 8 else k - 8
                else:
                    t, kk = w1_t[j], k
                c0 = kk * 256 + half * 128
                return t[:, c0 : c0 + 128]

            w2_t = []
            for k in range(NF):
                t = w2p.tile([128, H], bf16, tag=f"w2_{k}")
                w2_t.append(t)
                nc.sync.dma_start(out=t[:], in_=w2_d[k * 128 : (k + 1) * 128, :])

            for c in range(NCH):
                x_t = x0_t if c == 0 else load_x_chunk(c)

                # mm1 + swiglu, one (a, b) f-block pair at a time.
                act_t = []
                for j in range(NF):
                    ps_a = psp.tile([128, CHUNK], f32, tag="ps")
                    ps_b = psp.tile([128, CHUNK], f32, tag="ps")
                    for k in range(KH):
                        nc.tensor.matmul(
                            ps_a[:],
                            w1_lhsT(j, k, 0),
                            x_t[k],
                            start=(k == 0),
                            stop=(k == KH - 1),
                        )
                    for k in range(KH):
                        nc.tensor.matmul(
                            ps_b[:],
                            w1_lhsT(j, k, 1),
                            x_t[k],
                            start=(k == 0),
                            stop=(k == KH - 1),
                        )
                    tmp = tmpp.tile([128, CHUNK], f32, tag="tmp")
                    nc.scalar.activation(tmp[:], ps_a[:], SILU)
                    a = actp.tile([128, CHUNK], bf16, tag=f"act_{j}")
                    act_t.append(a)
                    nc.vector.tensor_mul(a[:], tmp[:], ps_b[:])

                # mm2: out[t, h]; n-outer so each PSUM bank drains while the
                # next accumulates (minimizes the end-of-kernel tail).
                for m in range(NT):
                    r0 = c * CHUNK + m * 128
                    for n in range(NHO):
                        po = psp.tile([128, 512], f32, tag="ps", name=f"po_{c}_{m}_{n}")
                        for k in range(NF):
                            nc.tensor.matmul(
                                po[:],
                                act_t[k][:, m * 128 : (m + 1) * 128],
                                w2_t[k][:, n * 512 : (n + 1) * 512],
                                start=(k == 0),
                                stop=(k == NF - 1),
                            )
                        osb = outp.tile([128, 512], bf16, tag="osb")
                        nc.scalar.copy(osb[:], po[:])
                        nc.sync.dma_start(
                            out=out_d[r0 : r0 + 128, n * 512 : (n + 1) * 512],
                            in_=osb[:],
                        )
    if not nc.is_finalized():
        nc.finalize()  # Bacc.finalize runs the lowering pipeline (sem split, alloc_regs)
    return nc


def _get_nc():
    if "nc" not in _CACHE:
        _CACHE["nc"] = _build()
    return _CACHE["nc"]


def _pack_x(xe: np.ndarray) -> np.ndarray:
    # [2048 tokens, 2048 hidden] -> [2048, 2048]: row (c*4+q)*128+p,
    # col kk*512+cc holds xT[(q*4+kk)*128+p, c*512+cc], so chunk c's
    # x tiles are 4 contiguous 4 KB-per-partition (512 KB) DMAs.
    return xe.reshape(NCH, CHUNK, 4, 4, 128).transpose(0, 2, 4, 3, 1).reshape(
        NCH * 4 * 128, 4 * CHUNK
    )


def _pack_w1(w1e: np.ndarray) -> np.ndarray:
    # [2048, 2816] -> [1408, 4096]: row jp*128+p, col k*256 + half*128 + c
    # holds w1[k*128+p, (jp + half*11)*128 + c] so pair jp's stationary
    # tiles for both swiglu halves of every k-tile are one contiguous
    # 8 KB-per-partition DMA.
    w1t = w1e.reshape(KH, 128, 2 * NF, 128).transpose(2, 1, 0, 3)
    return np.concatenate([w1t[:NF], w1t[NF:]], axis=3).reshape(NF * 128, KH * 256)


def kernel(permuted_hidden_states, num_tokens_per_expert, w1, w2):
    from concourse.bass_utils import run_bass_kernel_spmd

    x = np.asarray(permuted_hidden_states, dtype=np.float32)
    w1 = np.asarray(w1, dtype=np.float32)
    w2 = np.asarray(w2, dtype=np.float32)
    ntpe = np.asarray(num_tokens_per_expert)
    assert x.shape == (T_TOTAL, H) and w1.shape == (E, H, F2) and w2.shape == (E, F, H)
    # Reference semantics rely on the static equal split.
    assert np.all(ntpe == TPC), f"expected equal {TPC}-token splits, got {ntpe}"

    bf = ml_dtypes.bfloat16
    in_maps = []
    for e in range(E):
        xe = x[e * TPC : (e + 1) * TPC]
        in_maps.append(
            {
                "xq": np.ascontiguousarray(_pack_x(xe)).astype(bf),
                "w1p": np.ascontiguousarray(_pack_w1(w1[e])).astype(bf),
                "w2": np.ascontiguousarray(w2[e]).astype(bf),
            }
        )

    nc = _get_nc()
    res = run_bass_kernel_spmd(nc, in_maps, list(range(E)), trace=TRACE)
    LAST["exec_time_ns"] = res.exec_time_ns
    LAST["mean_exec_time_ns"] = res.mean_exec_time_ns
    LAST["profile_json"] = res.profile_json
    out = np.concatenate([res.results[i]["out"] for i in range(E)], axis=0)
    return np.ascontiguousarray(out.astype(np.float32))


# revision 22
# speedup vs baseline: 1.0171x; 1.0171x over previous
"""MoE expert FFN (swiglu) kernel for 8 trn2 NeuronCores.

Expert parallelism: 8 experts, one per core. Each core computes, for its
expert e:
    h   = x_e @ w1_e            # [2048, 2048] @ [2048, 2816]
    act = silu(h[:, :1408]) * h[:, 1408:]
    out = act @ w2_e            # [2048, 1408] @ [1408, 2048]

Tokens arrive pre-sorted by expert with equal counts (2048/expert), so
sharding is a static slice and the gather is a concat. No collectives.

Steady state (measured): back-to-back N=512 bf16 matmuls at 216 ns issue
spacing with LDWEIGHTS fully hidden -> 2112 MMs ~= 456 us dense PE floor.
Lower-precision matmul paths (fp8 DoubleRow 1.44x, int8) were ruled out
numerically: any <=2-matmul e4m3/int8 decomposition has >=2% rel err
against the 2e-2 gate (measured 6.5% all-e4m3, 2.1% all-int8), and
3-matmul hi/lo splits cost 2.08x bf16. So everything attacks the edges
around the dense stream (baseline 516.9 us -> ~477 us):

  1. w1 is host-repacked into 11 swiglu-pair slices [128, 16*256]
     (pair j holds the a-column-block j and b-column-block j+11 of all
     16 k-tiles, 1 MB each, contiguous 8 KB per partition). mm1 pair 0
     starts after ~1 MB of DMA instead of the whole 11.5 MB of w1
     (the original kernel idled 16 us, then stalled 17.6 us on w1
     residency). jp0 is further split in half (k-tiles 0-7 / 8-15).
  2. ALL DMA triggers live on the SP (sync) queue; the ACT queue holds
     only silu + PSUM-evacuation copies. HWDGE trigger instructions
     wait for their completion-semaphore lane's previous DMA before
     issuing, so triggers queued ahead of silu on ACT block the swiglu
     drain (measured: first silu at ~60 us -> PSUM-bank WAR stalled
     the PE 15.9 us and HAM re-throttled twice). Splitting the early
     triggers across SP+ACT rings was also measured 2-4 us WORSE (both
     rings share the same 16 SDMA engines).
  3. x is host-repacked per (chunk, quarter) so each chunk is 4
     contiguous [128, 2048] DMAs (4 KB/partition) instead of 16 small
     ones: fewer triggers, fewer sem-lane-reuse waits. Critical-path
     DMA order: jp0a, x-quarters 0-3, jp0b, jp1..jp10, w2 — pair-0's
     a-half consumes all four x quarters, jp0b only ~3.5 us later.
  4. PE pre-warm: 12 dummy matmuls on a zeroed tile right after the
     engine preamble, so the HAM clock-gate (1.2 -> 2.4 GHz after
     ~3.4 us of sustained activity) is released before the first real
     matmul, which is data-gated at ~13 us. The stream end is bound by
     delivery of chunk-0's x (2 MB) + jp0 (1 MB) at ~358 GB/s plus the
     dense 456 us stream; remaining losses are ~2 us of early
     delivery-bound gaps, ~3 us output tail, and ~7 us of framework
     semaphore-clear teardown inside the measured window.
  5. mm2 runs n-outer/k-inner so each PSUM bank finishes its 11-MM
     accumulation early and drains (ACT copy -> DMA) while the next
     bank accumulates; output staged as bf16 (host upcasts to f32,
     ~0.2% extra rounding, total rel err 4.4e-3 vs the 2e-2 gate).
     The out-DMA tail after the last matmul is ~3 us vs 14.4 us for
     the baseline's k-outer ordering with fp32 staging.

Device-side layout (bf16 compute, fp32 PSUM accumulation, bf16 out):
  mm1: out[f, t] tiles; lhsT = w1 pair-slice [128, 256] col pair
       (stationary), rhs = xT[h, t] (moving, N=512).
  swiglu: act_j = silu(ps_a) * ps_b via ACT(Silu) + DVE mul -> bf16.
  mm2: out[t, h]; lhsT = act[f, t] 128-col slices, rhs = w2[f, h]
       (moving, N=512). PSUM -> SBUF bf16 -> DMA out.
"""

import os
import sys

sys.path.insert(0, "/opt/trn_rl_repo")

import numpy as np
import ml_dtypes

E = 8             # experts == cores
T_TOTAL = 16384
H = 2048
F = 1408
F2 = 2 * F        # 2816
TPC = T_TOTAL // E  # 2048 tokens per core
CHUNK = 512
NCH = TPC // CHUNK          # 4 chunks
KH = H // 128               # 16 contraction tiles for mm1
NF = F // 128               # 11 f-blocks per half (a / b)
NT = CHUNK // 128           # 4 m-tiles per chunk in mm2
NHO = H // 512              # 4 output column blocks
NWARM = 12                  # dummy matmuls to release the HAM clock gate

_CACHE = {}

# Optional knobs read by test.py (not used by the grading harness).
TRACE = os.environ.get("BASS_TRACE_KERNEL", "0") == "1"
LAST = {}


def _build():
    from concourse import bacc, tile, mybir

    bf16 = mybir.dt.bfloat16
    f32 = mybir.dt.float32
    SILU = mybir.ActivationFunctionType.Silu

    # Bacc (not plain Bass): its lowering pipeline splits multi-sem waits
    # into EventSemaphore pairs — TRN2 allows at most 1 wait per instruction.
    nc = bacc.Bacc()
    # x packed by (chunk, quarter): row (c*4+q)*128+p, col kk*512+cc holds
    # xT[(q*4+kk)*128+p, c*512+cc] — each chunk is 4 contiguous 512 KB DMAs.
    xq_d = nc.declare_dram_parameter("xq", [NCH * 4 * 128, 4 * CHUNK], bf16, isOutput=False)
    # w1 packed by swiglu pair: row jp*128+p, col k*256 + half*128 + c
    w1_d = nc.declare_dram_parameter("w1p", [NF * 128, KH * 256], bf16, isOutput=False)
    w2_d = nc.declare_dram_parameter("w2", [F, H], bf16, isOutput=False)
    out_d = nc.declare_dram_parameter("out", [TPC, H], bf16, isOutput=True)

    with tile.TileContext(nc) as tc:
        with (
            tc.tile_pool(name="w1p", bufs=1) as w1p,
            tc.tile_pool(name="w2p", bufs=1) as w2p,
            tc.tile_pool(name="xp", bufs=2) as xp,
            tc.tile_pool(name="actp", bufs=1) as actp,
            tc.tile_pool(name="tmpp", bufs=2) as tmpp,
            tc.tile_pool(name="outp", bufs=4) as outp,
            tc.tile_pool(name="warmp", bufs=1) as warmp,
            tc.tile_pool(name="psp", bufs=8, space="PSUM") as psp,
        ):
            # PE warm-up: zero tile, then dummy matmuls with no DMA deps so
            # the PE is busy during the startup DMA window and the HAM gate
            # opens (K=8/8) before the first real matmul.
            wz = warmp.tile([128, 512], bf16, tag="wz")
            nc.vector.memset(wz[:], 0)
            ps_w = psp.tile([128, 512], f32, tag="ps", name="warm")
            for i in range(NWARM):
                nc.tensor.matmul(
                    ps_w[:],
                    wz[:, 0:128],
                    wz[:],
                    start=(i == 0),
                    stop=(i == NWARM - 1),
                )

            # SP queue order: jp0 first (gates the first real matmul), then
            # chunk 0's x quarters, then the rest of w1, then w2 (needed
            # ~100 us in, must not steal startup bandwidth).
            # x stays on the SP HWDGE ring: routing it via SWDGE (gpsimd/Q7)
            # was measured ~9 us WORSE — Q7 descriptor emission is slower
            # than the HWDGE ring and the early x quarters arrived late.
            def load_x_chunk(c):
                qt = []
                for q in range(4):
                    t = xp.tile([128, 4 * CHUNK], bf16, tag=f"x_{q}", name=f"x_{c}_{q}")
                    qt.append(t)
                    r = (c * 4 + q) * 128
                    nc.sync.dma_start(out=t[:], in_=xq_d[r : r + 128, :])
                # x_t[k] is a 512-col view into quarter k//4
                return [qt[k // 4][:, (k % 4) * CHUNK : (k % 4 + 1) * CHUNK]
                        for k in range(KH)]

            # jp0 is split in half (k-tiles 0-7 / 8-15) so the first real
            # matmul is gated by ~1 MB of DMA, not 2.5 MB. Everything stays
            # on the SP queue: an A/B
            # splitting these across SP+ACT rings measured 2-4 us WORSE
            # (both rings share the same 16 SDMA engines), and triggers on
            # ACT risk stalling the swiglu drain (the v2 lesson).
            # Pair-0's a-half k-loop consumes all 4 x quarters, so they all
            # precede jp0b (the b-half weights, not needed until ~3.5 us
            # after the a-half starts completing).
            w1_0a = w1p.tile([128, 8 * 256], bf16, tag="w1_0a")
            nc.sync.dma_start(out=w1_0a[:], in_=w1_d[0:128, 0 : 8 * 256])

            x0_t = load_x_chunk(0)

            w1_0b = w1p.tile([128, 8 * 256], bf16, tag="w1_0b")
            nc.sync.dma_start(out=w1_0b[:], in_=w1_d[0:128, 8 * 256 :])

            w1_t = [None]
            for j in range(1, NF):
                t = w1p.tile([128, KH * 256], bf16, tag=f"w1_{j}")
                w1_t.append(t)
                nc.sync.dma_start(out=t[:], in_=w1_d[j * 128 : (j + 1) * 128, :])

            def w1_lhsT(j, k, half):
                if j == 0:
                    t = w1_0a if k < 8 else w1_0b
                    kk = k if k < 8 else k - 8
                else:
                    t, kk = w1_t[j], k
                c0 = kk * 256 + half * 128
                return t[:, c0 : c0 + 128]

            w2_t = []
            for k in range(NF):
                t = w2p.tile([128, H], bf16, tag=f"w2_{k}")
                w2_t.append(t)
                nc.sync.dma_start(out=t[:], in_=w2_d[k * 128 : (k + 1) * 128, :])

            for c in range(NCH):
                x_t = x0_t if c == 0 else load_x_chunk(c)

                # mm1 + swiglu, one (a, b) f-block pair at a time.
                act_t = []
                for j in range(NF):
                    ps_a = psp.tile([128, CHUNK], f32, tag="ps")
                    ps_b = psp.tile([128, CHUNK], f32, tag="ps")
                    for k in range(KH):
                        nc.tensor.matmul(
                            ps_a[:],
                            w1_lhsT(j, k, 0),
                            x_t[k],
                            start=(k == 0),
                            stop=(k == KH - 1),
                        )
                    for k in range(KH):
                        nc.tensor.matmul(
                            ps_b[:],
                            w1_lhsT(j, k, 1),
                            x_t[k],
                            start=(k == 0),
                            stop=(k == KH - 1),
                        )
                    tmp = tmpp.tile([128, CHUNK], f32, tag="tmp")
                    nc.scalar.activation(tmp[:], ps_a[:], SILU)
                    a = actp.tile([128, CHUNK], bf16, tag=f"act_{j}")
                    act_t.append(a)
                    nc.vector.tensor_mul(a[:], tmp[:], ps_b[:])

                # mm2: out[t, h]; n-outer so each PSUM bank drains while the
                # next accumulates (minimizes the end-of-kernel tail).
                for m in range(NT):
                    r0 = c * CHUNK + m * 128
                    for n in range(NHO):
                        po = psp.tile([128, 512], f32, tag="ps", name=f"po_{c}_{m}_{n}")
                        for k in range(NF):
                            nc.tensor.matmul(
                                po[:],
                                act_t[k][:, m * 128 : (m + 1) * 128],
                                w2_t[k][:, n * 512 : (n + 1) * 512],
                                start=(k == 0),
                                stop=(k == NF - 1),
                            )
                        osb = outp.tile([128, 512], bf16, tag="osb")
                        nc.scalar.copy(osb[:], po[:])
                        nc.sync.dma_start(
                            out=out_d[r0 : r0 + 128, n * 512 : (n + 1) * 512],
                            in_=osb[:],
                        )
    if not nc.is_finalized():
        nc.finalize()  # Bacc.finalize runs the lowering pipeline (sem split, alloc_regs)
    return nc


def _get_nc():
    if "nc" not in _CACHE:
        _CACHE["nc"] = _build()
    return _CACHE["nc"]


def _pack_x(xe: np.ndarray) -> np.ndarray:
    # [2048 tokens, 2048 hidden] -> [2048, 2048]: row (c*4+q)*128+p,
    # col kk*512+cc holds xT[(q*4+kk)*128+p, c*512+cc], so chunk c's
    # x tiles are 4 contiguous 4 KB-per-partition (512 KB) DMAs.
    return xe.reshape(NCH, CHUNK, 4, 4, 128).transpose(0, 2, 4, 3, 1).reshape(
        NCH * 4 * 128, 4 * CHUNK
    )


def _pack_w1(w1e: np.ndarray) -> np.ndarray:
    # [2048, 2816] -> [1408, 4096]: row jp*128+p, col k*256 + half*128 + c
    # holds w1[k*128+p, (jp + half*11)*128 + c] so pair jp's stationary
    # tiles for both swiglu halves of every k-tile are one contiguous
    # 8 KB-per-partition DMA.
    w1t = w1e.reshape(KH, 128, 2 * NF, 128).transpose(2, 1, 0, 3)
    return np.concatenate([w1t[:NF], w1t[NF:]], axis=3).reshape(NF * 128, KH * 256)


def kernel(permuted_hidden_states, num_tokens_per_expert, w1, w2):
    from concourse.bass_utils import run_bass_kernel_spmd

    x = np.asarray(permuted_hidden_states, dtype=np.float32)
    w1 = np.asarray(w1, dtype=np.float32)
    w2 = np.asarray(w2, dtype=np.float32)
    ntpe = np.asarray(num_tokens_per_expert)
    assert x.shape == (T_TOTAL, H) and w1.shape == (E, H, F2) and w2.shape == (E, F, H)
    # Reference semantics rely on the static equal split.
    assert np.all(ntpe == TPC), f"expected equal {TPC}-token splits, got {ntpe}"

    bf = ml_dtypes.bfloat16
    in_maps = []
    for e in range(E):
        xe = x[e * TPC : (e + 1) * TPC]
        in_maps.append(
            {
                "xq": np.ascontiguousarray(_pack_x(xe)).astype(bf),
                "w1p": np.ascontiguousarray(_pack_w1(w1[e])).astype(bf),
                "w2": np.ascontiguousarray(w2[e]).astype(bf),
            }
        )

    nc = _get_nc()
    res = run_bass_kernel_spmd(nc, in_maps, list(range(E)), trace=TRACE)
    LAST["exec_time_ns"] = res.exec_time_ns
    LAST["mean_exec_time_ns"] = res.mean_exec_time_ns
    LAST["profile_json"] = res.profile_json
    out = np.concatenate([res.results[i]["out"] for i in range(E)], axis=0)
    return np.ascontiguousarray(out.astype(np.float32))
